# revision 19
# baseline (speedup 1.0000x reference)
"""Trainium2 Bass kernel for batched filtfilt band-pass filtering (tensorpac-style).

Math: scipy-style filtfilt with FIR taps b is (exactly) a single convolution of
the odd-extended input with the autocorrelation of b, evaluated on the interior:

    out[n] = sum_d A[d] * ext[P + n + d],   d in [-(t-1), t-1]
    A[d]   = sum_i b[i] * b[i+d]            (t = effective tap count)

provided padlen P >= t-1 (true here: P = 512, t <= 513). The left "lfilter_zi"
constant extension and the right-edge extension of the backward pass never reach
the retained [P, P+L) window, so the equivalence is exact (verified to 1e-16).

A's tails are products of Hamming-window tails and decay fast: truncating to
lags |d| <= L_k with per-band tail l2 <= 3e-3 (vs the 2e-2 budget; fp16 noise
alone is 3.3e-4) shrinks the banded support. Structural gains only are taken:
the block count Q_k is fixed from the tolerance, then L_k is RAISED back to
the largest value 64*(Q_k-1) the geometry still covers, so every band keeps
the most accuracy its block count allows. This drops whole 128-blocks from
big bands (Q 9->7, 7->6, 4->3, two 3->2) and pulls the four smallest bands
under L <= 32, where FOUR bands ride in one shared 128x128 Toeplitz block
(32 output rows each, s=32): each group then needs just 4 matmuls - one per
32-position sub-offset, rhs from the E96/E/E32/E64 shifted ext copies - in
place of the 16 the four singles would need. 264 matmuls/core -> 224.

Device mapping (per core, sequence-parallel over 8 cores):
  - each core owns 2048 output positions x all 128 batches; its input is a
    (3072, 128) slice of ext^T (position-major) covering the 2x512 halo,
    shipped fp16 in the SBUF-native [partition, h-block, batch] layout.
    The shifted variants (rows 32/64/96 + 128h + p) are sliced on the HOST
    and shipped as separate inputs: building them on device with SBUF->SBUF
    DMAs contends with the PE's rhs reads (matmuls measurably run at 2x
    duration under a concurrent build) and with the DVE drain writes.
  - out tiles (128 rows x 4 pos-blocks x 128 batches) accumulate in fp32 PSUM
    via K=128 fp16 matmuls: lhsT = 128x128 banded-Toeplitz blocks of A
    (host-precomputed fp16 constants), rhs = 512-wide slices of ext^T.
  - every item runs GROUP-OUTER (PSUM drains right after each group's Q
    matmuls; LDWEIGHTS is issued per-matmul by the lowering anyway, so
    qi-outer weight amortization buys nothing). The item order interleaves
    drain-heavy items (the quad, Q=2 bands) between big-Q bands so the
    DVE/ACT drain stream never runs a deficit against the PE stream.
  - PSUM tiles drain via a DVE/ACT split copy that also casts to fp16; out
    ships in tapered multi-slot chunks (one contiguous DMA each, alternating
    rings); the final item ships per group so the kernel tail is one 128KB
    flush, not 512KB.
  - dummy warm-up matmuls run while the first inputs land so the PE HAM
    clock-gate is released before real work starts.
"""

import os

import numpy as np

import concourse.mybir as mybir
from concourse import bacc
from concourse.tile import TileContext
from concourse.bass_utils import run_bass_kernel_spmd

F32 = mybir.dt.float32
F16 = mybir.dt.float16

B = 128          # batch
L = 16384        # sequence length
P = 512          # padlen (= TAPS - 1)
NB = 20          # bands
N_CORES = 8
LC = L // N_CORES            # 2048 output positions per core
GROUPS = LC // 512           # 4 groups of 512 positions
EXT_ROWS = LC + 2 * P        # 3072 ext rows per core (halo included)
H_E = EXT_ROWS // 128        # 24 aligned 128-row blocks
H_SH = (EXT_ROWS - 128) // 128   # 23 blocks for the shifted copies
N_WARM = 4                   # dummy matmuls to warm the PE HAM during input DMA
TRUNC_TOL = 3e-3             # per-band autocorr tail l2 budget (rel)

LAST_RESULT = None  # BassKernelResults of the most recent run (for test harness)

_program_cache: dict = {}


def _acorr_full(b):
    """Autocorrelation on the full lag grid [-P, P] (float64)."""
    t = len(b)
    a = np.correlate(b, b, mode="full")  # 2t-1, center t-1
    a_full = np.zeros(2 * P + 1, np.float64)
    a_full[P - (t - 1): P + t] = a
    return a_full


def _band_plan(kernels: np.ndarray):
    """Per-band truncated lag support L and block geometry.

    Block q covers ext rows m = n0 + P - s + 128q + kk (kk = partition), so
    diagonal d = 128q + kk - s - r. Coverage of d in [-L, L] for every
    r in [0,128) requires s >= L and s <= 128Q - 128 - L; s is a multiple
    of 64 (s % 128 == 64 sources the rhs from the 64-shifted ext copy).
    Q is fixed from the truncation tolerance, then L raised to 64*(Q-1),
    the largest lag the Q-block geometry covers. Bands whose tolerance
    support is <= 32 are quadable: four bands share one block at 32 output
    rows each (s = 32, d = kk - 32 - r' in [-63, 95] covers |d| <= 32).
    """
    plan = []
    for k in range(kernels.shape[0]):
        nz = np.nonzero(kernels[k])[0]
        t = int(nz[-1]) + 1 if nz.size else 1
        assert t - 1 <= P, f"band {k}: taps {t} exceed padlen {P}"
        b = kernels[k][:t].astype(np.float64)
        a = np.correlate(b, b, mode="full")
        c0 = t - 1
        nrm = np.linalg.norm(a) + 1e-300
        L_min = t - 1
        for Ltry in range(t - 2, -1, -1):
            tail = np.concatenate([a[: c0 - Ltry], a[c0 + Ltry + 1:]])
            if np.linalg.norm(tail) / nrm <= TRUNC_TOL:
                L_min = Ltry
            else:
                break
        quadable = L_min <= 32
        if quadable:
            Lv = min(t - 1, 32)
            s, q = 32, 1
        else:
            s_min = 64 * ((L_min + 63) // 64) if L_min > 0 else 0
            q = (s_min + L_min + 128 + 127) // 128
            Lv = min(t - 1, 64 * (q - 1))
            s = 64 * ((Lv + 63) // 64) if Lv > 0 else 0
            assert s >= Lv and s <= 128 * q - 128 - Lv, (k, Lv, s, q)
        use64 = (s % 128) == 64
        h_base = (P - 64 - s) // 128 if use64 else (P - s) // 128
        assert h_base >= 0
        plan.append((t, Lv, q, s, use64, h_base, quadable))
    # quads hold exactly 4 bands; demote leftovers to plain Q=2 singles
    quadbands = [k for k in range(len(plan)) if plan[k][6]]
    for k in quadbands[4 * (len(quadbands) // 4):]:
        t = plan[k][0]
        plan[k] = (t, min(t - 1, 64), 2, 64, True, (P - 128) // 128, False)
    return plan


def _build_items(plan):
    """Group bands into schedule items (normal bands and 32-row quads) and
    order them so the DVE/ACT drain stream keeps pace with the PE stream.

    Drain model (per 2048-col slot: ~1.86us; per-slot matmul: Q*4*216ns):
    a quad produces 4 slots off 16 matmuls (slack -4us), Q=2 bands -0.13us,
    Q>=4 bands +1.6..+4.2us. Start on an aligned (E-only) Q=3 band (the
    shifted ext copies land a few us into the matmul stream), interleave
    bigs with Q=2 bands, park the quad after the third big so its sources
    (built over three DMA rings after E lands) are ready, and end on an
    aligned Q=3 band whose groups ship individually."""
    quadbands = [k for k in range(len(plan)) if plan[k][6]]
    normals = [k for k in range(len(plan)) if not plan[k][6]]
    items = []
    assert len(quadbands) % 4 == 0  # _band_plan demoted any leftovers
    for qi in range(0, len(quadbands), 4):
        items.append({"kind": "quad", "bands": tuple(quadbands[qi: qi + 4]),
                      "nslots": 4, "nblk": 1})
    for k in normals:
        items.append({"kind": "normal", "band": k, "nslots": 1,
                      "nblk": plan[k][2]})

    def q_of(it):
        return plan[it["band"]][2] if it["kind"] == "normal" else 0

    def aligned(it):
        return it["kind"] == "normal" and not plan[it["band"]][4]

    q3s = sorted([it for it in items if it["kind"] == "normal"
                  and q_of(it) == 3 and aligned(it)],
                 key=lambda it: -plan[it["band"]][1])
    assert len(q3s) >= 2, "need aligned Q=3 bands for first/last"
    first, last = q3s[0], q3s[1]
    rest = [it for it in items if it is not first and it is not last]
    bigs = sorted([it for it in rest if it["kind"] == "normal" and q_of(it) >= 4],
                  key=lambda it: -q_of(it))
    quads = [it for it in rest if it["kind"] == "quad"]
    q2s = [it for it in rest if it["kind"] == "normal" and q_of(it) == 2]
    mids = [it for it in rest if it["kind"] == "normal" and q_of(it) == 3]
    order = [first]
    li = 0
    for bi, bg in enumerate(bigs):
        order.append(bg)
        if bi >= 2 and quads:
            order.append(quads.pop(0))
        elif li < len(q2s):
            order.append(q2s[li]); li += 1
    order.extend(quads)
    for md in mids:
        if li < len(q2s):
            order.append(q2s[li]); li += 1
        order.append(md)
    order.extend(q2s[li:])
    order.append(last)
    assert len(order) == len(items)
    so = bo = 0
    for it in order:
        it["slot"] = so
        it["block_off"] = bo
        so += it["nslots"]
        bo += it["nblk"]
    return order, so, bo


def _toeplitz_blocks(kernels: np.ndarray, plan, items, nblk):
    """Stacked lhsT blocks in SBUF-native layout: (128, NBLK, 128) fp16,
    [kk, block, r] with the contraction dim kk on axis 0, laid out in
    schedule order so the constant stream is a few contiguous DMAs."""
    out = np.zeros((128, nblk, 128), np.float16)
    kk = np.arange(128)[:, None]

    def banded(k, dmat):
        t, Lv = plan[k][0], plan[k][1]
        a_full = _acorr_full(kernels[k][:t].astype(np.float64))
        valid = (dmat >= -Lv) & (dmat <= Lv)
        return np.where(valid, a_full[np.clip(dmat + P, 0, 2 * P)], 0.0)

    for it in items:
        o = it["block_off"]
        if it["kind"] == "normal":
            k = it["band"]
            s = plan[k][3]
            rr = np.arange(128)[None, :]
            for q in range(it["nblk"]):
                d = 128 * q - s + kk - rr
                out[:, o + q, :] = banded(k, d).astype(np.float16)
        else:
            blk = np.zeros((128, 128))
            rq = np.arange(32)[None, :]
            for i, k in enumerate(it["bands"]):
                blk[:, 32 * i: 32 * i + 32] = banded(k, kk - 32 - rq)
            out[:, o, :] = blk.astype(np.float16)
    return out


def _out_chunks(items):
    """Tapered out-DMA chunking over schedule items: leading items group into
    ~2-slot chunks (fewer ~0.6us triggers; a quad ships as its own 4-slot
    chunk), trailing items ship solo the moment they drain; the last item
    ships per-group inside the main loop."""
    n = len(items)
    chunks = []
    cur = []
    cur_slots = 0
    for idx, it in enumerate(items[:-1]):
        if it["kind"] == "quad":
            if cur:
                chunks.append(cur)
            chunks.append([idx])
            cur, cur_slots = [], 0
            continue
        solo_zone = idx >= n - 6
        cur.append(idx)
        cur_slots += it["nslots"]
        if solo_zone or cur_slots >= 2:
            chunks.append(cur)
            cur, cur_slots = [], 0
    if cur:
        chunks.append(cur)
    chunks.append([n - 1])  # final item: per-group ship
    return chunks


def _build_program(plan_key):
    """Compile the SPMD program for a given block structure. Cached."""
    if plan_key in _program_cache:
        return _program_cache[plan_key]

    plan = list(plan_key)
    items, nslots, nblk = _build_items(plan)
    assert nslots == NB
    chunks = _out_chunks(items)
    chunk_of_item = {}
    for ci, idxs in enumerate(chunks):
        for idx in idxs:
            chunk_of_item[idx] = ci

    # lhs constant stream graduation (item-range boundaries -> block ranges)
    n_it = len(items)
    lhs_cuts = sorted({0, 1, 2, min(4, n_it), min(7, n_it), n_it})

    nc = bacc.Bacc("TRN2", target_bir_lowering=False, debug=False,
                   num_devices=N_CORES)
    need_quad = any(it["kind"] == "quad" for it in items)
    # host-permuted ext^T slices: [p, h, b] fp16 (SBUF-native layout);
    # extNN holds ext rows (NN + 128h + p)
    ext_in = nc.declare_dram_parameter("ext", [128, H_E, B], F16, isOutput=False)
    e64_in = nc.declare_dram_parameter("ext64", [128, H_SH, B], F16,
                                       isOutput=False)
    if need_quad:
        e32_in = nc.declare_dram_parameter("ext32", [128, H_SH, B], F16,
                                           isOutput=False)
        e96_in = nc.declare_dram_parameter("ext96", [128, H_SH, B], F16,
                                           isOutput=False)
    lhs_in = nc.declare_dram_parameter("lhs", [128, nblk, 128], F16,
                                       isOutput=False)
    out_t = nc.declare_dram_parameter("out", [NB, 128, GROUPS * 512], F16,
                                      isOutput=True)

    with TileContext(nc) as tc:
        with (
            tc.tile_pool(name="consts", bufs=1) as cpool,
            tc.tile_pool(name="psum", bufs=8, space="PSUM") as ppool,
            tc.tile_pool(name="ostage", bufs=6) as opool,
        ):
            E = cpool.tile([128, H_E * 128], F16)
            E64 = cpool.tile([128, H_SH * 128], F16)
            if need_quad:
                E32 = cpool.tile([128, H_SH * 128], F16)
                E96 = cpool.tile([128, H_SH * 128], F16)
            Lw = cpool.tile([128, nblk * 128], F16)
            warm = cpool.tile([128, 256], F16)
            wps = ppool.tile([128, 512], F32, tag="ps")

            # PE warm-up during the input DMAs: harmless matmuls on a zeroed
            # tile keep the HAM busy window alive so real matmuls start warm.
            # memset on DVE: nc.any would pick GpSimd, whose multi-us engine
            # cold-start delays the whole warm-up chain.
            nc.vector.memset(warm[:], 0.0)
            for w in range(N_WARM):
                nc.tensor.matmul(wps[:, 0:256], warm[:, :128], warm[:],
                                 start=True, stop=True)

            # E in 2 chunks: the first covers the h-blocks the first two
            # items' g=0 matmuls touch (each chunk costs ~128 descriptor
            # issues regardless of width, so fewer chunks finish sooner);
            # then the 64-shift (feeds schedule slot ~2). The 32/96 shifts
            # only feed the mid-schedule quad and ride the ACT ring after
            # the lhs constants.
            e_flat = ext_in[:].rearrange("p h b -> p (h b)")
            chunk0 = 12 * 128  # covers item0's g0/g1 + item1's g0
            nc.sync.dma_start(out=E[:, 0:chunk0], in_=e_flat[:, 0:chunk0])
            nc.sync.dma_start(out=E[:, chunk0:], in_=e_flat[:, chunk0:])
            nc.sync.dma_start(out=E64[:], in_=e64_in[:].rearrange("p h b -> p (h b)"))

            # constants are pre-ordered schedule-major on the host, so the
            # ~1.3 MB stream is a few contiguous graduated DMAs on the ACT
            # HWDGE ring. Graduation matters because a DMA completes as one
            # unit: each chunk must land before the MM stream reaches its
            # first block, so early chunks are small.
            for lo, hi in zip(lhs_cuts[:-1], lhs_cuts[1:]):
                oa = items[lo]["block_off"]
                ob_ = (items[hi]["block_off"] if hi < n_it else nblk)
                nc.scalar.dma_start(
                    out=Lw[:, oa * 128: ob_ * 128].rearrange(
                        "kk (i r) -> kk i r", r=128
                    ),
                    in_=lhs_in[:, oa:ob_, :],
                )
            # E32/E96 feed only the mid-schedule quad: their triggers are
            # deferred into the item loop (after the first odd out-chunk
            # ship) so their SBUF writes don't contend with the early
            # matmuls' rhs reads; see delayed_loads below.

            # staging tiles for the tapered multi-slot out-DMAs
            chunk_tiles = {}
            chunk_slot0 = {}
            for ci, idxs in enumerate(chunks):
                ns = sum(items[idx]["nslots"] for idx in idxs)
                chunk_slot0[ci] = items[idxs[0]]["slot"]
                chunk_tiles[ci] = opool.tile(
                    [128, ns * GROUPS * 512], F16, name="obc",
                    tag=f"obc{ns}", bufs=(2 if ns > 1 else 3),
                )

            def drain(ps, ob, base):
                # split the PSUM drain across DVE and ACT so neither engine
                # gates the PSUM bank turnaround; 352/160 balances the
                # measured per-col rates (DVE 1.25ns, ACT 2.8ns)
                nc.vector.tensor_copy(ob[:, base: base + 352], ps[:, 0:352])
                nc.scalar.copy(ob[:, base + 352: base + 512], ps[:, 352:512])

            # defer the quad's sources until the schedule point after the
            # second out-chunk completes its drains (still ~10us before the
            # quad's first matmul), unless the quad sits earlier than that
            quad_idx = next((i for i, it in enumerate(items)
                             if it["kind"] == "quad"), None)
            delay_after = None
            if need_quad:
                delay_after = chunks[1][-1] if chunks[1][-1] < quad_idx else None
                if delay_after is None:
                    nc.scalar.dma_start(
                        out=E32[:], in_=e32_in[:].rearrange("p h b -> p (h b)"))
                    nc.scalar.dma_start(
                        out=E96[:], in_=e96_in[:].rearrange("p h b -> p (h b)"))

            last_idx = len(items) - 1
            for idx, it in enumerate(items):
                ci = chunk_of_item[idx]
                ob = chunk_tiles[ci]
                obase = (it["slot"] - chunk_slot0[ci]) * GROUPS * 512
                o = it["block_off"]
                if it["kind"] == "normal":
                    k = it["band"]
                    _t, _L, q_cnt, _s, use64, h_base, _qd = plan[k]
                    src = E64 if use64 else E
                    h_max = H_SH if use64 else H_E
                    for g in range(GROUPS):
                        ps = ppool.tile([128, 512], F32, name="ps", tag="ps")
                        for qq in range(q_cnt):
                            h0 = 4 * g + h_base + qq
                            assert 0 <= h0 and h0 + 4 <= h_max, (k, g, qq, h0)
                            nc.tensor.matmul(
                                ps[:],
                                Lw[:, (o + qq) * 128: (o + qq + 1) * 128],
                                src[:, h0 * 128: h0 * 128 + 512],
                                start=(qq == 0),
                                stop=(qq == q_cnt - 1),
                            )
                        base = obase + g * 512
                        drain(ps, ob, base)
                        if idx == last_idx:
                            # final item ships per-group on alternating rings
                            # so the kernel's last HBM flush is 128KB; the
                            # very last group splits across BOTH rings (64
                            # partitions each) to halve the tail's
                            # descriptor-issue chain
                            rng = slice(g * 512, g * 512 + 512)
                            if g == GROUPS - 1:
                                nc.sync.dma_start(
                                    out=out_t[it["slot"], 0:64, rng],
                                    in_=ob[0:64, base: base + 512])
                                nc.scalar.dma_start(
                                    out=out_t[it["slot"], 64:128, rng],
                                    in_=ob[64:128, base: base + 512])
                            else:
                                eng = nc.sync if g % 2 == 0 else nc.scalar
                                eng.dma_start(
                                    out=out_t[it["slot"], :, rng],
                                    in_=ob[:, base: base + 512],
                                )
                else:
                    # quad: one shared lhsT block, 4 bands x 32 rows; four
                    # matmuls per group, one per 32-position sub-offset,
                    # rhs from the four shifted ext copies (s = 32)
                    w = Lw[:, o * 128: (o + 1) * 128]
                    srcs = ((E96, 3), (E, 4), (E32, 4), (E64, 4))
                    for g in range(GROUPS):
                        for ss, (src, hb) in enumerate(srcs):
                            h0 = hb + 4 * g
                            ps = ppool.tile([128, 512], F32, name="ps", tag="ps")
                            nc.tensor.matmul(ps[:], w,
                                             src[:, h0 * 128: h0 * 128 + 512],
                                             start=True, stop=True)
                            drain(ps, ob, obase + ss * GROUPS * 512 + g * 512)
                # ship each completed chunk as ONE contiguous DMA (out_t is
                # slot-major; the host unscrambles), alternating rings
                # chunk-by-chunk. Keep the partition dim outermost on BOTH
                # sides of the AP - a leading free dim over SBUF partitions
                # generates descriptors the DGE cannot execute.
                if idx == chunks[ci][-1] and idx != last_idx:
                    s0 = chunk_slot0[ci]
                    ns = sum(items[j]["nslots"] for j in chunks[ci])
                    eng = nc.sync if ci % 2 == 0 else nc.scalar
                    eng.dma_start(
                        out=out_t[s0: s0 + ns].rearrange("i p f -> p i f"),
                        in_=ob[:].rearrange("p (i f) -> p i f", i=ns),
                    )
                if delay_after is not None and idx == delay_after:
                    nc.scalar.dma_start(
                        out=E32[:], in_=e32_in[:].rearrange("p h b -> p (h b)"))
                    nc.scalar.dma_start(
                        out=E96[:], in_=e96_in[:].rearrange("p h b -> p (h b)"))

    nc.compile()
    _program_cache[plan_key] = (nc, items)
    return nc, items


def _maybe_register_trace_hook():
    """Best-effort registration of the axon NTFF profile hook (profiling only;
    harmless no-op if unavailable)."""
    try:
        import sys
        import types

        import antenv

        if getattr(antenv, "axon_hooks", None) is not None:
            return
        from trn_agent_boot.trn_boot import _ntff_profile_via_ctypes

        hooks = types.ModuleType("antenv.axon_hooks")
        hook = _ntff_profile_via_ctypes("/opt/axon/libaxon_pjrt.so")
        hooks.get_axon_ntff_profile_hook = lambda: hook
        hooks.set_axon_ntff_profile_hook = lambda h: None
        antenv.axon_hooks = hooks
        sys.modules["antenv.axon_hooks"] = hooks
    except Exception:
        pass


def kernel(x: np.ndarray, kernels: np.ndarray, padlen) -> np.ndarray:
    global LAST_RESULT
    x = np.asarray(x, dtype=np.float32)
    kernels = np.asarray(kernels, dtype=np.float32)
    assert x.shape == (B, 1, L) and kernels.shape[0] == NB
    assert int(padlen) == P

    plan = _band_plan(kernels)
    plan_key = tuple(plan)
    nc, items = _build_program(plan_key)

    nblk = sum(it["nblk"] for it in items)
    lhs = np.ascontiguousarray(_toeplitz_blocks(kernels, plan, items, nblk))

    # odd extension + transpose to position-major (ext^T), fp16
    x2d = x[:, 0, :]
    left = 2.0 * x2d[:, :1] - x2d[:, 1: P + 1][:, ::-1]
    right = 2.0 * x2d[:, -1:] - x2d[:, -P - 1: -1][:, ::-1]
    ext_t = np.concatenate([left, x2d, right], axis=1).T.astype(np.float16)

    need_quad = any(it["kind"] == "quad" for it in items)
    in_maps = []
    for c in range(N_CORES):
        # SBUF-native layout [p, h, b]: ext row (s0 + 128h + p) -> [p, h]
        def shifted(s0, H):
            sl = ext_t[c * LC + s0: c * LC + s0 + H * 128]
            return np.ascontiguousarray(
                sl.reshape(H, 128, B).transpose(1, 0, 2)
            )

        m = {"ext": shifted(0, H_E), "ext64": shifted(64, H_SH), "lhs": lhs}
        if need_quad:
            m["ext32"] = shifted(32, H_SH)
            m["ext96"] = shifted(96, H_SH)
        in_maps.append(m)

    trace = bool(os.environ.get("KERNEL_TRACE"))
    if trace:
        _maybe_register_trace_hook()
    res = run_bass_kernel_spmd(nc, in_maps, list(range(N_CORES)), trace=trace)
    LAST_RESULT = res

    out = np.empty((B, 1, NB, L), np.float32)
    for c in range(N_CORES):
        dev = res.results[c]["out"].astype(np.float32)
        dev = dev.reshape(NB, 128, GROUPS, 4, 128)  # [slot, r, g, j, b]
        cl = slice(c * LC, (c + 1) * LC)
        for it in items:
            s = it["slot"]
            if it["kind"] == "normal":
                # dev[s, r, g, j, b] -> out[b, 0, k, c*LC + 512g + 128j + r]
                out[:, 0, it["band"], cl] = (
                    dev[s].transpose(3, 1, 2, 0).reshape(B, LC)
                )
            else:
                # slot s+ss = sub-offset ss; rows 32i:32i+32 = band i of the
                # quad; position = 512g + 128j + 32*ss + r'
                quad = dev[s: s + 4].reshape(4, 4, 32, GROUPS, 4, 128)
                # [ss, i, r', g, j, b] -> [i, b, g, j, ss, r']
                quad = quad.transpose(1, 5, 3, 4, 0, 2).reshape(4, B, LC)
                for i, k in enumerate(it["bands"]):
                    out[:, 0, k, cl] = quad[i]
    return out


# revision 23
# speedup vs baseline: 1.0317x; 1.0317x over previous
"""Trainium2 Bass kernel for batched filtfilt band-pass filtering (tensorpac-style).

Math: scipy-style filtfilt with FIR taps b is (exactly) a single convolution of
the odd-extended input with the autocorrelation of b, evaluated on the interior:

    out[n] = sum_d A[d] * ext[P + n + d],   d in [-(t-1), t-1]
    A[d]   = sum_i b[i] * b[i+d]            (t = effective tap count)

provided padlen P >= t-1 (true here: P = 512, t <= 513). The left "lfilter_zi"
constant extension and the right-edge extension of the backward pass never reach
the retained [P, P+L) window, so the equivalence is exact (verified to 1e-16).

A's tails are products of Hamming-window tails and decay fast: truncating to
lags |d| <= L_k with per-band tail l2 <= 3e-3 (vs the 2e-2 budget; fp16 noise
alone is 3.3e-4) shrinks the banded support. Structural gains only are taken:
the block count Q_k is fixed from the tolerance, then L_k is RAISED back to
the largest value 64*(Q_k-1) the geometry still covers, so every band keeps
the most accuracy its block count allows. This drops whole 128-blocks from
big bands (Q 9->7, 7->6, 4->3, two 3->2) and pulls the four smallest bands
under L <= 32, where FOUR bands ride in one shared 128x128 Toeplitz block
(32 output rows each, s=32): each group then needs just 4 matmuls - one per
32-position sub-offset, rhs from the E96/E/E32/E64 shifted ext copies - in
place of the 16 the four singles would need. 264 matmuls/core -> 224.

Device mapping (per core, sequence-parallel over 8 cores):
  - each core owns 2048 output positions x all 128 batches; its input is a
    (3072, 128) slice of ext^T (position-major) covering the 2x512 halo,
    shipped fp16 in the SBUF-native [partition, h-block, batch] layout.
    The shifted variants (rows 32/64/96 + 128h + p) are sliced on the HOST
    and shipped as separate inputs: building them on device with SBUF->SBUF
    DMAs contends with the PE's rhs reads (matmuls measurably run at 2x
    duration under a concurrent build) and with the DVE drain writes.
  - out tiles (128 rows x 4 pos-blocks x 128 batches) accumulate in fp32 PSUM
    via K=128 fp16 matmuls: lhsT = 128x128 banded-Toeplitz blocks of A
    (host-precomputed fp16 constants), rhs = 512-wide slices of ext^T.
  - every item runs GROUP-OUTER (PSUM drains right after each group's Q
    matmuls; LDWEIGHTS is issued per-matmul by the lowering anyway, so
    qi-outer weight amortization buys nothing). The item order interleaves
    drain-heavy items (the quad, Q=2 bands) between big-Q bands so the
    DVE/ACT drain stream never runs a deficit against the PE stream.
  - PSUM tiles drain via a DVE/ACT split copy that also casts to fp16; out
    ships in tapered multi-slot chunks (one contiguous DMA each, alternating
    rings); the final item ships per group so the kernel tail is one 128KB
    flush, not 512KB.
  - dummy warm-up matmuls run while the first inputs land so the PE HAM
    clock-gate is released before real work starts.
"""

import os

import numpy as np

import concourse.mybir as mybir
from concourse import bacc
from concourse.tile import TileContext
from concourse.bass_utils import run_bass_kernel_spmd

F32 = mybir.dt.float32
F16 = mybir.dt.float16

B = 128          # batch
L = 16384        # sequence length
P = 512          # padlen (= TAPS - 1)
NB = 20          # bands
N_CORES = 8
LC = L // N_CORES            # 2048 output positions per core
GROUPS = LC // 512           # 4 groups of 512 positions
EXT_ROWS = LC + 2 * P        # 3072 ext rows per core (halo included)
H_E = EXT_ROWS // 128        # 24 aligned 128-row blocks
H_SH = (EXT_ROWS - 128) // 128   # 23 blocks for the shifted copies
N_WARM = 4                   # dummy matmuls to warm the PE HAM during input DMA
TRUNC_TOL = 3e-3             # per-band autocorr tail l2 budget (rel)

LAST_RESULT = None  # BassKernelResults of the most recent run (for test harness)

_program_cache: dict = {}


def _acorr_full(b):
    """Autocorrelation on the full lag grid [-P, P] (float64)."""
    t = len(b)
    a = np.correlate(b, b, mode="full")  # 2t-1, center t-1
    a_full = np.zeros(2 * P + 1, np.float64)
    a_full[P - (t - 1): P + t] = a
    return a_full


def _band_plan(kernels: np.ndarray):
    """Per-band truncated lag support L and block geometry.

    Block q covers ext rows m = n0 + P - s + 128q + kk (kk = partition), so
    diagonal d = 128q + kk - s - r. Coverage of d in [-L, L] for every
    r in [0,128) requires s >= L and s <= 128Q - 128 - L; s is a multiple
    of 64 (s % 128 == 64 sources the rhs from the 64-shifted ext copy).
    Q is fixed from the truncation tolerance, then L raised to 64*(Q-1),
    the largest lag the Q-block geometry covers. Bands whose tolerance
    support is <= 32 are quadable: four bands share one block at 32 output
    rows each (s = 32, d = kk - 32 - r' in [-63, 95] covers |d| <= 32).
    """
    plan = []
    for k in range(kernels.shape[0]):
        nz = np.nonzero(kernels[k])[0]
        t = int(nz[-1]) + 1 if nz.size else 1
        assert t - 1 <= P, f"band {k}: taps {t} exceed padlen {P}"
        b = kernels[k][:t].astype(np.float64)
        a = np.correlate(b, b, mode="full")
        c0 = t - 1
        nrm = np.linalg.norm(a) + 1e-300
        L_min = t - 1
        for Ltry in range(t - 2, -1, -1):
            tail = np.concatenate([a[: c0 - Ltry], a[c0 + Ltry + 1:]])
            if np.linalg.norm(tail) / nrm <= TRUNC_TOL:
                L_min = Ltry
            else:
                break
        quadable = L_min <= 32
        if quadable:
            Lv = min(t - 1, 32)
            s, q = 32, 1
        else:
            s_min = 64 * ((L_min + 63) // 64) if L_min > 0 else 0
            q = (s_min + L_min + 128 + 127) // 128
            Lv = min(t - 1, 64 * (q - 1))
            s = 64 * ((Lv + 63) // 64) if Lv > 0 else 0
            assert s >= Lv and s <= 128 * q - 128 - Lv, (k, Lv, s, q)
        use64 = (s % 128) == 64
        h_base = (P - 64 - s) // 128 if use64 else (P - s) // 128
        assert h_base >= 0
        plan.append((t, Lv, q, s, use64, h_base, quadable))
    # quads hold exactly 4 bands; demote leftovers to plain Q=2 singles
    quadbands = [k for k in range(len(plan)) if plan[k][6]]
    for k in quadbands[4 * (len(quadbands) // 4):]:
        t = plan[k][0]
        plan[k] = (t, min(t - 1, 64), 2, 64, True, (P - 128) // 128, False)
    return plan


def _build_items(plan):
    """Group bands into schedule items (normal bands and 32-row quads) and
    order them so the DVE/ACT drain stream keeps pace with the PE stream.

    Drain model (per 2048-col slot: ~1.86us; per-slot matmul: Q*4*216ns):
    a quad produces 4 slots off 16 matmuls (slack -4us), Q=2 bands -0.13us,
    Q>=4 bands +1.6..+4.2us. Start on an aligned (E-only) Q=3 band (the
    shifted ext copies land a few us into the matmul stream), interleave
    bigs with Q=2 bands, park the quad after the third big so its sources
    (built over three DMA rings after E lands) are ready, and end on an
    aligned Q=3 band whose groups ship individually."""
    quadbands = [k for k in range(len(plan)) if plan[k][6]]
    normals = [k for k in range(len(plan)) if not plan[k][6]]
    items = []
    assert len(quadbands) % 4 == 0  # _band_plan demoted any leftovers
    for qi in range(0, len(quadbands), 4):
        items.append({"kind": "quad", "bands": tuple(quadbands[qi: qi + 4]),
                      "nslots": 4, "nblk": 1})
    for k in normals:
        items.append({"kind": "normal", "band": k, "nslots": 1,
                      "nblk": plan[k][2]})

    def q_of(it):
        return plan[it["band"]][2] if it["kind"] == "normal" else 0

    def aligned(it):
        return it["kind"] == "normal" and not plan[it["band"]][4]

    q3s = sorted([it for it in items if it["kind"] == "normal"
                  and q_of(it) == 3 and aligned(it)],
                 key=lambda it: -plan[it["band"]][1])
    assert len(q3s) >= 2, "need aligned Q=3 bands for first/last"
    first, last = q3s[0], q3s[1]
    rest = [it for it in items if it is not first and it is not last]
    bigs = sorted([it for it in rest if it["kind"] == "normal" and q_of(it) >= 4],
                  key=lambda it: -q_of(it))
    quads = [it for it in rest if it["kind"] == "quad"]
    q2s = [it for it in rest if it["kind"] == "normal" and q_of(it) == 2]
    mids = [it for it in rest if it["kind"] == "normal" and q_of(it) == 3]
    order = [first]
    li = 0
    for bi, bg in enumerate(bigs):
        order.append(bg)
        if bi >= 2 and quads:
            order.append(quads.pop(0))
        elif li < len(q2s):
            order.append(q2s[li]); li += 1
    order.extend(quads)
    for md in mids:
        if li < len(q2s):
            order.append(q2s[li]); li += 1
        order.append(md)
    order.extend(q2s[li:])
    order.append(last)
    assert len(order) == len(items)
    so = bo = 0
    for it in order:
        it["slot"] = so
        it["block_off"] = bo
        so += it["nslots"]
        bo += it["nblk"]
    return order, so, bo


def _toeplitz_blocks(kernels: np.ndarray, plan, items, nblk):
    """Stacked lhsT blocks in SBUF-native layout: (128, NBLK, 128) fp16,
    [kk, block, r] with the contraction dim kk on axis 0, laid out in
    schedule order so the constant stream is a few contiguous DMAs."""
    out = np.zeros((128, nblk, 128), np.float16)
    kk = np.arange(128)[:, None]

    def banded(k, dmat):
        t, Lv = plan[k][0], plan[k][1]
        a_full = _acorr_full(kernels[k][:t].astype(np.float64))
        valid = (dmat >= -Lv) & (dmat <= Lv)
        return np.where(valid, a_full[np.clip(dmat + P, 0, 2 * P)], 0.0)

    for it in items:
        o = it["block_off"]
        if it["kind"] == "normal":
            k = it["band"]
            s = plan[k][3]
            rr = np.arange(128)[None, :]
            for q in range(it["nblk"]):
                d = 128 * q - s + kk - rr
                out[:, o + q, :] = banded(k, d).astype(np.float16)
        else:
            blk = np.zeros((128, 128))
            rq = np.arange(32)[None, :]
            for i, k in enumerate(it["bands"]):
                blk[:, 32 * i: 32 * i + 32] = banded(k, kk - 32 - rq)
            out[:, o, :] = blk.astype(np.float16)
    return out


def _out_chunks(items):
    """Tapered out-DMA chunking over schedule items: leading items group into
    ~2-slot chunks (fewer ~0.6us triggers; a quad ships as its own 4-slot
    chunk), trailing items ship solo the moment they drain; the last item
    ships per-group inside the main loop."""
    n = len(items)
    chunks = []
    cur = []
    cur_slots = 0
    for idx, it in enumerate(items[:-1]):
        if it["kind"] == "quad":
            if cur:
                chunks.append(cur)
            chunks.append([idx])
            cur, cur_slots = [], 0
            continue
        solo_zone = idx >= n - 6
        cur.append(idx)
        cur_slots += it["nslots"]
        if solo_zone or cur_slots >= 2:
            chunks.append(cur)
            cur, cur_slots = [], 0
    if cur:
        chunks.append(cur)
    chunks.append([n - 1])  # final item: per-group ship
    return chunks


def _build_program(plan_key):
    """Compile the SPMD program for a given block structure. Cached."""
    if plan_key in _program_cache:
        return _program_cache[plan_key]

    plan = list(plan_key)
    items, nslots, nblk = _build_items(plan)
    assert nslots == NB
    chunks = _out_chunks(items)
    chunk_of_item = {}
    for ci, idxs in enumerate(chunks):
        for idx in idxs:
            chunk_of_item[idx] = ci

    # lhs constant stream graduation (item-range boundaries -> block ranges)
    n_it = len(items)
    lhs_cuts = sorted({0, 1, 2, min(4, n_it), min(7, n_it), n_it})

    nc = bacc.Bacc("TRN2", target_bir_lowering=False, debug=False,
                   num_devices=N_CORES)
    need_quad = any(it["kind"] == "quad" for it in items)
    # host-permuted ext^T slices: [p, h, b] fp16 (SBUF-native layout);
    # extNN holds ext rows (NN + 128h + p)
    ext_in = nc.declare_dram_parameter("ext", [128, H_E, B], F16, isOutput=False)
    e64_in = nc.declare_dram_parameter("ext64", [128, H_SH, B], F16,
                                       isOutput=False)
    if need_quad:
        e32_in = nc.declare_dram_parameter("ext32", [128, H_SH, B], F16,
                                           isOutput=False)
        e96_in = nc.declare_dram_parameter("ext96", [128, H_SH, B], F16,
                                           isOutput=False)
    lhs_in = nc.declare_dram_parameter("lhs", [128, nblk, 128], F16,
                                       isOutput=False)
    out_t = nc.declare_dram_parameter("out", [NB, 128, GROUPS * 512], F16,
                                      isOutput=True)

    with TileContext(nc) as tc:
        with (
            tc.tile_pool(name="consts", bufs=1) as cpool,
            tc.tile_pool(name="psum", bufs=8, space="PSUM") as ppool,
            tc.tile_pool(name="ostage", bufs=6) as opool,
        ):
            E = cpool.tile([128, H_E * 128], F16)
            E64 = cpool.tile([128, H_SH * 128], F16)
            if need_quad:
                E32 = cpool.tile([128, H_SH * 128], F16)
                E96 = cpool.tile([128, H_SH * 128], F16)
            Lw = cpool.tile([128, nblk * 128], F16)
            warm = cpool.tile([128, 256], F16)
            wps = ppool.tile([128, 512], F32, tag="ps")

            # PE warm-up during the input DMAs: harmless matmuls on a zeroed
            # tile keep the HAM busy window alive so real matmuls start warm.
            # memset on DVE: nc.any would pick GpSimd, whose multi-us engine
            # cold-start delays the whole warm-up chain.
            nc.vector.memset(warm[:], 0.0)
            for w in range(N_WARM):
                nc.tensor.matmul(wps[:, 0:256], warm[:, :128], warm[:],
                                 start=True, stop=True)

            # E in 2 chunks: the first covers the h-blocks the first two
            # items' g=0 matmuls touch (each chunk costs ~128 descriptor
            # issues regardless of width, so fewer chunks finish sooner);
            # then the 64-shift (feeds schedule slot ~2). The 32/96 shifts
            # only feed the mid-schedule quad and ride the ACT ring after
            # the lhs constants.
            e_flat = ext_in[:].rearrange("p h b -> p (h b)")
            chunk0 = 12 * 128  # covers item0's g0/g1 + item1's g0
            nc.sync.dma_start(out=E[:, 0:chunk0], in_=e_flat[:, 0:chunk0])
            nc.sync.dma_start(out=E[:, chunk0:], in_=e_flat[:, chunk0:])
            nc.sync.dma_start(out=E64[:], in_=e64_in[:].rearrange("p h b -> p (h b)"))

            # constants are pre-ordered schedule-major on the host, so the
            # ~1.3 MB stream is a few contiguous graduated DMAs on the ACT
            # HWDGE ring. Graduation matters because a DMA completes as one
            # unit: each chunk must land before the MM stream reaches its
            # first block, so early chunks are small.
            for lo, hi in zip(lhs_cuts[:-1], lhs_cuts[1:]):
                oa = items[lo]["block_off"]
                ob_ = (items[hi]["block_off"] if hi < n_it else nblk)
                nc.scalar.dma_start(
                    out=Lw[:, oa * 128: ob_ * 128].rearrange(
                        "kk (i r) -> kk i r", r=128
                    ),
                    in_=lhs_in[:, oa:ob_, :],
                )
            if need_quad:
                # E32/E96 feed only the mid-schedule quad; they ride the ACT
                # ring behind the lhs constants (deferring them further into
                # the schedule was tried and bought nothing — the early
                # matmul slowdown comes from the E/E64/lhs streams)
                nc.scalar.dma_start(out=E32[:],
                                    in_=e32_in[:].rearrange("p h b -> p (h b)"))
                nc.scalar.dma_start(out=E96[:],
                                    in_=e96_in[:].rearrange("p h b -> p (h b)"))

            # staging tiles for the tapered multi-slot out-DMAs
            chunk_tiles = {}
            chunk_slot0 = {}
            for ci, idxs in enumerate(chunks):
                ns = sum(items[idx]["nslots"] for idx in idxs)
                chunk_slot0[ci] = items[idxs[0]]["slot"]
                chunk_tiles[ci] = opool.tile(
                    [128, ns * GROUPS * 512], F16, name="obc",
                    tag=f"obc{ns}", bufs=(2 if ns > 1 else 3),
                )

            def drain(ps, ob, base):
                # split the PSUM drain across DVE and ACT so neither engine
                # gates the PSUM bank turnaround; 352/160 balances the
                # measured per-col rates (DVE 1.25ns, ACT 2.8ns)
                nc.vector.tensor_copy(ob[:, base: base + 352], ps[:, 0:352])
                nc.scalar.copy(ob[:, base + 352: base + 512], ps[:, 352:512])

            last_idx = len(items) - 1
            for idx, it in enumerate(items):
                ci = chunk_of_item[idx]
                ob = chunk_tiles[ci]
                obase = (it["slot"] - chunk_slot0[ci]) * GROUPS * 512
                o = it["block_off"]
                if it["kind"] == "normal":
                    k = it["band"]
                    _t, _L, q_cnt, _s, use64, h_base, _qd = plan[k]
                    src = E64 if use64 else E
                    h_max = H_SH if use64 else H_E
                    for g in range(GROUPS):
                        ps = ppool.tile([128, 512], F32, name="ps", tag="ps")
                        for qq in range(q_cnt):
                            h0 = 4 * g + h_base + qq
                            assert 0 <= h0 and h0 + 4 <= h_max, (k, g, qq, h0)
                            nc.tensor.matmul(
                                ps[:],
                                Lw[:, (o + qq) * 128: (o + qq + 1) * 128],
                                src[:, h0 * 128: h0 * 128 + 512],
                                start=(qq == 0),
                                stop=(qq == q_cnt - 1),
                            )
                        base = obase + g * 512
                        drain(ps, ob, base)
                        if idx == last_idx:
                            # final item ships per-group on alternating rings
                            # so the kernel's last HBM flush is 128KB (NOTE:
                            # splitting the last group across both rings was
                            # tried and costs ~2us extra teardown — both
                            # rings then have to quiesce at the tail)
                            eng = nc.sync if g % 2 == 0 else nc.scalar
                            eng.dma_start(
                                out=out_t[it["slot"], :, g * 512: g * 512 + 512],
                                in_=ob[:, base: base + 512],
                            )
                else:
                    # quad: one shared lhsT block, 4 bands x 32 rows; four
                    # matmuls per group, one per 32-position sub-offset,
                    # rhs from the four shifted ext copies (s = 32)
                    w = Lw[:, o * 128: (o + 1) * 128]
                    srcs = ((E96, 3), (E, 4), (E32, 4), (E64, 4))
                    for g in range(GROUPS):
                        for ss, (src, hb) in enumerate(srcs):
                            h0 = hb + 4 * g
                            ps = ppool.tile([128, 512], F32, name="ps", tag="ps")
                            nc.tensor.matmul(ps[:], w,
                                             src[:, h0 * 128: h0 * 128 + 512],
                                             start=True, stop=True)
                            drain(ps, ob, obase + ss * GROUPS * 512 + g * 512)
                # ship each completed chunk as ONE contiguous DMA (out_t is
                # slot-major; the host unscrambles), alternating rings
                # chunk-by-chunk. Keep the partition dim outermost on BOTH
                # sides of the AP - a leading free dim over SBUF partitions
                # generates descriptors the DGE cannot execute.
                if idx == chunks[ci][-1] and idx != last_idx:
                    s0 = chunk_slot0[ci]
                    ns = sum(items[j]["nslots"] for j in chunks[ci])
                    eng = nc.sync if ci % 2 == 0 else nc.scalar
                    eng.dma_start(
                        out=out_t[s0: s0 + ns].rearrange("i p f -> p i f"),
                        in_=ob[:].rearrange("p (i f) -> p i f", i=ns),
                    )


    nc.compile()
    _program_cache[plan_key] = (nc, items)
    return nc, items


def _maybe_register_trace_hook():
    """Best-effort registration of the axon NTFF profile hook (profiling only;
    harmless no-op if unavailable)."""
    try:
        import sys
        import types

        import antenv

        if getattr(antenv, "axon_hooks", None) is not None:
            return
        from trn_agent_boot.trn_boot import _ntff_profile_via_ctypes

        hooks = types.ModuleType("antenv.axon_hooks")
        hook = _ntff_profile_via_ctypes("/opt/axon/libaxon_pjrt.so")
        hooks.get_axon_ntff_profile_hook = lambda: hook
        hooks.set_axon_ntff_profile_hook = lambda h: None
        antenv.axon_hooks = hooks
        sys.modules["antenv.axon_hooks"] = hooks
    except Exception:
        pass


def kernel(x: np.ndarray, kernels: np.ndarray, padlen) -> np.ndarray:
    global LAST_RESULT
    x = np.asarray(x, dtype=np.float32)
    kernels = np.asarray(kernels, dtype=np.float32)
    assert x.shape == (B, 1, L) and kernels.shape[0] == NB
    assert int(padlen) == P

    plan = _band_plan(kernels)
    plan_key = tuple(plan)
    nc, items = _build_program(plan_key)

    nblk = sum(it["nblk"] for it in items)
    lhs = np.ascontiguousarray(_toeplitz_blocks(kernels, plan, items, nblk))

    # odd extension + transpose to position-major (ext^T), fp16
    x2d = x[:, 0, :]
    left = 2.0 * x2d[:, :1] - x2d[:, 1: P + 1][:, ::-1]
    right = 2.0 * x2d[:, -1:] - x2d[:, -P - 1: -1][:, ::-1]
    ext_t = np.concatenate([left, x2d, right], axis=1).T.astype(np.float16)

    need_quad = any(it["kind"] == "quad" for it in items)
    in_maps = []
    for c in range(N_CORES):
        # SBUF-native layout [p, h, b]: ext row (s0 + 128h + p) -> [p, h]
        def shifted(s0, H):
            sl = ext_t[c * LC + s0: c * LC + s0 + H * 128]
            return np.ascontiguousarray(
                sl.reshape(H, 128, B).transpose(1, 0, 2)
            )

        m = {"ext": shifted(0, H_E), "ext64": shifted(64, H_SH), "lhs": lhs}
        if need_quad:
            m["ext32"] = shifted(32, H_SH)
            m["ext96"] = shifted(96, H_SH)
        in_maps.append(m)

    trace = bool(os.environ.get("KERNEL_TRACE"))
    if trace:
        _maybe_register_trace_hook()
    res = run_bass_kernel_spmd(nc, in_maps, list(range(N_CORES)), trace=trace)
    LAST_RESULT = res

    out = np.empty((B, 1, NB, L), np.float32)
    for c in range(N_CORES):
        dev = res.results[c]["out"].astype(np.float32)
        dev = dev.reshape(NB, 128, GROUPS, 4, 128)  # [slot, r, g, j, b]
        cl = slice(c * LC, (c + 1) * LC)
        for it in items:
            s = it["slot"]
            if it["kind"] == "normal":
                # dev[s, r, g, j, b] -> out[b, 0, k, c*LC + 512g + 128j + r]
                out[:, 0, it["band"], cl] = (
                    dev[s].transpose(3, 1, 2, 0).reshape(B, LC)
                )
            else:
                # slot s+ss = sub-offset ss; rows 32i:32i+32 = band i of the
                # quad; position = 512g + 128j + 32*ss + r'
                quad = dev[s: s + 4].reshape(4, 4, 32, GROUPS, 4, 128)
                # [ss, i, r', g, j, b] -> [i, b, g, j, ss, r']
                quad = quad.transpose(1, 5, 3, 4, 0, 2).reshape(4, B, LC)
                for i, k in enumerate(it["bands"]):
                    out[:, 0, k, cl] = quad[i]
    return out
